# revision 3
# baseline (speedup 1.0000x reference)
"""DetectionLoss Trainium2 kernel (8-core data parallel).

Reference computation (per sample):
  decode 1176 pred boxes -> IoU vs 64 targets -> argmax over preds ->
  smooth-l1 on matched boxes + CE on matched class logits + BCE(conf, pos mask).
Output: scalar loss = (5*box + cls + conf) / 512.

Layout: groups of 2 samples; partitions = 2 x 64 targets; free dim = preds.
PE builds rank-2 "broadcast" matrices (i1 = bx2-tx1, i2 = tx2-bx1, S = ab+at,
wb, hb) in fp32r; ACT relu-evacuates; intersection dx = min(relu(i1),
relu(i2), wb, wt); score = dx*dy/(ab+at) which is monotone in IoU, so the
hardware max/max_index (first-occurrence) reproduces jnp.argmax exactly,
including all-zero-IoU ties.
"""

import numpy as np
from contextlib import ExitStack

import concourse.bass as bass
import concourse.mybir as mybir
from concourse import bacc, tile
from concourse.bass_utils import run_bass_kernel_spmd
from concourse.masks import make_identity

F32 = mybir.dt.float32
F32R = mybir.dt.float32r
I32 = mybir.dt.int32
U32 = mybir.dt.uint32
OP = mybir.AluOpType
AF = mybir.ActivationFunctionType
AX = mybir.AxisListType

B, N, T, C = 512, 1176, 64, 4
NCORES = 8
BC = B // NCORES          # samples per core = 64
NG = BC // 2              # groups of 2 samples = 32
NJ = 10                   # n tiles of 128 (padded)
NN = NJ * 128             # 1280
NTAIL = N - 9 * 128       # 24 valid rows in tile j=9
CHUNKS = [(0, 512), (512, 512), (1024, 256)]
IMG_W, IMG_H = 1472.0, 832.0
LN16 = float(np.log(np.float64(16.0)))
SQRT_HALF = float(np.sqrt(np.float64(0.5)))

# pred-row quantity order (PE rhs rows); const row value per quantity
#   q0 I1x = bx2 - tx1   rhs=BX2,  const=-1, lhsT2=tx1
#   q1 I2x = tx2 - bx1   rhs=NBX1, const=+1, lhsT2=tx2
#   q2 I1y = by2 - ty1   rhs=BY2,  const=-1, lhsT2=ty1
#   q3 I2y = ty2 - by1   rhs=NBY1, const=+1, lhsT2=ty2
#   q4 SAB = ab + at     rhs=AB,   const=+1, lhsT2=at
#   q5 WBt = wb          rhs=WB,   const=0
#   q6 HBt = hb          rhs=HB,   const=0
NQ = 7
QCONST = [-1.0, 1.0, -1.0, 1.0, 1.0, 0.0, 0.0]

USE_GPSIMD = False
DEBUG = False
import os
STAGE = int(os.environ.get("KSTAGE", "4"))
KSUB = int(os.environ.get("KSUB", "9"))


def r32(ap):
    return ap.bitcast(F32R)


def build_kernel():
    nc = bacc.Bacc(
        "TRN2",
        target_bir_lowering=False,
        debug=False,
        enable_asserts=False,
        num_devices=NCORES,
    )
    pred_d = nc.dram_tensor("predictions", [BC, N, 9], F32, kind="ExternalInput").ap()
    tb_d = nc.dram_tensor("target_boxes", [BC, T, 4], F32, kind="ExternalInput").ap()
    tc_d = nc.dram_tensor("target_classes", [BC, T], I32, kind="ExternalInput").ap()
    ir_d = nc.dram_tensor(
        "initrows", [3, NQ * NN + NQ * 128], F32R, kind="ExternalInput"
    ).ap()
    out_d = nc.dram_tensor("out", [3], F32, kind="ExternalOutput").ap()
    dbg_d = nc.dram_tensor("dbg", [128, NG], F32, kind="ExternalOutput").ap() if DEBUG else None
    dbgs_d = nc.dram_tensor("dbgS", [NG, 128, NN], F32, kind="ExternalOutput").ap() if DEBUG else None

    with tile.TileContext(nc) as tcx:
        with ExitStack() as ctx:
            emit(ctx, tcx, pred_d, tb_d, tc_d, ir_d, out_d, dbg_d, dbgs_d)
    nc.compile()
    return nc


def emit(ctx, tcx, pred_d, tb_d, tc_d, ir_d, out_d, dbg_d, dbgs_d):
    nc = tcx.nc
    tp = lambda name, bufs, **kw: ctx.enter_context(
        tcx.tile_pool(name=name, bufs=bufs, **kw)
    )

    const_p = tp("const", 1)
    big_p = tp("big", 1)
    rows_p = tp("rows", 1)
    work_p = tp("work", 2)
    small_p = tp("small", 3)
    psA_p = tp("psA", 1, space="PSUM")      # 7 quantity banks
    psB_p = tp("psB", 1, space="PSUM")      # prep transposes / MB / gathers

    vec = nc.vector
    act = nc.scalar
    gps = nc.gpsimd if USE_GPSIMD else nc.vector

    # ---------------- constants ----------------
    ident = const_p.tile([128, 128], F32, tag="ident")
    make_identity(nc, ident[:, :])
    ones1 = const_p.tile([1, 128], F32, tag="ones1")
    nc.vector.memset(ones1[:, :], 1.0)
    onescol = const_p.tile([128, 1], F32, tag="onescol")
    nc.vector.memset(onescol[:, :], 1.0)
    njcol_i = const_p.tile([128, NJ], I32, tag="njcol_i")
    nc.gpsimd.iota(njcol_i[:, :], pattern=[[128, NJ]], base=0, channel_multiplier=1)
    njcol = const_p.tile([128, NJ], F32, tag="njcol")
    vec.tensor_copy(njcol[:, :], njcol_i[:, :])
    cbias = const_p.tile([128, 4], F32, tag="cbias")
    nc.vector.memset(cbias[:, 0:1], LN16)
    nc.vector.memset(cbias[:, 1:2], -IMG_W / 2)
    nc.vector.memset(cbias[:, 2:3], -IMG_H / 2)
    nc.vector.memset(cbias[:, 3:4], -1.0)
    b_ln16, b_wneg, b_hneg, b_neg1 = (cbias[:, i : i + 1] for i in range(4))  # noqa

    # ---------------- stage 0: loads ----------------
    # X[p, s, j, k] = pred[s, j*128+p, k]; pad rows zeroed, conf col -> -100
    X = big_p.tile([128, BC, NJ, 9], F32, tag="X")
    nc.vector.memset(X[:, :, 9, :], 0.0)
    nc.vector.memset(X[:, :, 9, 4], -100.0)
    nsplit = 2
    sw = BC // nsplit
    for i in range(nsplit):
        s0 = i * sw
        for j in range(NJ):
            pw = 128 if j < 9 else NTAIL
            eng = [nc.sync, nc.gpsimd, nc.scalar][(i * NJ + j) % 3]
            eng.dma_start(
                X[0:pw, s0 : s0 + sw, j, :],
                pred_d[s0 : s0 + sw, j * 128 : j * 128 + pw, :].rearrange(
                    "s p k -> p s k"
                ),
            )

    # targets: TGTC[p=(s,t), g, c], TCI[p, g]
    TGTC = big_p.tile([128, NG, 4], F32, tag="TGTC")
    nc.sync.dma_start(TGTC[:, :, :], tb_d.rearrange("(g s) t c -> (s t) g c", s=2))
    TCI = big_p.tile([128, NG], I32, tag="TCI")
    nc.sync.dma_start(TCI[:, :], tc_d.rearrange("(g s) t -> (s t) g", s=2))
    TCF = big_p.tile([128, NG], F32, tag="TCF")
    vec.tensor_copy(TCF[:, :], TCI[:, :])

    # ---------------- stage 0: decode ----------------
    # DQ[p, s, q, j]: q in (BX2, NBX1, BY2, NBY1, AB, WB, HB)
    DQ = big_p.tile([128, BC, NQ, NJ], F32, tag="DQ")
    WHX = big_p.tile([128, BC, NJ], F32, tag="WHX")
    WHY = big_p.tile([128, BC, NJ], F32, tag="WHY")
    CXY = big_p.tile([128, 2, BC, NJ], F32, tag="CXY")
    act.activation(WHX[:, :, :], X[:, :, :, 2], AF.Exp, bias=b_ln16, scale=1.0)
    act.activation(WHY[:, :, :], X[:, :, :, 3], AF.Exp, bias=b_ln16, scale=1.0)
    act.activation(
        CXY[:, 0, :, :], X[:, :, :, 0], AF.Identity, bias=b_wneg, scale=IMG_W
    )
    act.activation(
        CXY[:, 1, :, :], X[:, :, :, 1], AF.Identity, bias=b_hneg, scale=IMG_H
    )
    vec.tensor_tensor(DQ[:, :, 0, :], CXY[:, 0, :, :], WHX[:, :, :], OP.add)
    vec.tensor_tensor(DQ[:, :, 1, :], WHX[:, :, :], CXY[:, 0, :, :], OP.subtract)
    vec.tensor_tensor(DQ[:, :, 2, :], CXY[:, 1, :, :], WHY[:, :, :], OP.add)
    vec.tensor_tensor(DQ[:, :, 3, :], WHY[:, :, :], CXY[:, 1, :, :], OP.subtract)
    # q4 = AB (for SAB), q5 = WB, q6 = HB -- must match QCONST/lq mapping
    vec.tensor_tensor(DQ[:, :, 5, :], DQ[:, :, 0, :], DQ[:, :, 1, :], OP.add)
    vec.tensor_tensor(DQ[:, :, 6, :], DQ[:, :, 2, :], DQ[:, :, 3, :], OP.add)
    vec.tensor_tensor(DQ[:, :, 4, :], DQ[:, :, 5, :], DQ[:, :, 6, :], OP.mult)

    if STAGE == 1:
        o1 = small_p.tile([3, 1], F32, tag="o1")
        vec.tensor_copy(o1[:, :], DQ[0:3, 0, 0, 0:1])
        nc.sync.dma_start(out_d[:].rearrange("(x o) -> x o", o=1), o1[:, :])
        return

    # target-derived
    WT = big_p.tile([128, NG], F32, tag="WT")
    HT = big_p.tile([128, NG], F32, tag="HT")
    AT = big_p.tile([128, NG], F32, tag="AT")
    vec.tensor_tensor(WT[:, :], TGTC[:, :, 2], TGTC[:, :, 0], OP.subtract)
    vec.tensor_tensor(HT[:, :], TGTC[:, :, 3], TGTC[:, :, 1], OP.subtract)
    vec.tensor_tensor(AT[:, :], WT[:, :], HT[:, :], OP.mult)
    # TRI rows feed lq row2 in q-order: (tx1, tx2, ty1, ty2, at)
    TRI = big_p.tile([128, NG, 5], F32, tag="TRI")
    vec.tensor_copy(
        TRI[:, :, 0:4].rearrange("p g (b a) -> p g b a", a=2),
        TGTC[:, :, :].rearrange("p g (a b) -> p g b a", b=2),
    )
    vec.tensor_copy(TRI[:, :, 4], AT[:, :])

    # persistent lhsT / rhs row tiles (double-buffered by hand so const rows
    # are written once per buffer)
    prs, lqs = [], []
    for half in range(2):
        pr = rows_p.tile([3, NQ, NN], F32R, tag=f"pr{half}", name=f"pr{half}")
        lq = rows_p.tile([3, NQ, 128], F32R, tag=f"lq{half}", name=f"lq{half}")
        nc.sync.dma_start(
            pr[:, :, :], ir_d[:, 0 : NQ * NN].rearrange("x (q n) -> x q n", q=NQ)
        )
        nc.sync.dma_start(
            lq[:, :, :],
            ir_d[:, NQ * NN :].rearrange("x (q n) -> x q n", q=NQ),
        )
        prs.append(pr)
        lqs.append(lq)

    # accumulators
    ACCB = big_p.tile([128, 8], F32, tag="ACCB")     # smooth-l1 partials
    ACCC = big_p.tile([128, 2], F32, tag="ACCC")     # cls partials
    nc.vector.memset(ACCB[:, :], 0.0)
    nc.vector.memset(ACCC[:, :], 0.0)
    CNT = big_p.tile([128, NJ, BC], F32, tag="CNT")  # match counts
    GALL = big_p.tile([128, NG, 16], F32, tag="GALL")

    # ---------------- per-group matching ----------------
    for g in range(NG):
        pr, lq = prs[g % 2], lqs[g % 2]

        # prep: pred rows for both samples, target rows (PE transpose ->
        # SBUF stage -> reshape DMA into row layout)
        for s in range(2):
            tpq = psB_p.tile([NQ * NJ, 128], F32, tag="tp")
            nc.tensor.transpose(
                tpq[:, :],
                DQ[:, 2 * g + s, :, :].rearrange("p q j -> p (q j)"),
                ident[:, :],
            )
            stq = small_p.tile([NQ * NJ, 128], F32R, tag="stq")
            vec.tensor_copy(stq[:, :], tpq[:, :])
            nc.sync.dma_start(
                pr[s : s + 1, :, :].rearrange("one q (j p) -> one (q j) p", p=128),
                stq[:, :],
            )
        tpt = psB_p.tile([5, 128], F32, tag="tp")
        nc.tensor.transpose(tpt[:, :], TRI[:, g, :], ident[:, :])
        stt_ = small_p.tile([5, 128], F32R, tag="stt")
        vec.tensor_copy(stt_[:, :], tpt[:, :])
        nc.sync.dma_start(lq[2:3, 0:5, :], stt_[:, :])

        S = work_p.tile([128, NN], F32, tag="S")
        if KSUB == 1:
            continue
        for ci, (c0, cw) in enumerate(CHUNKS):
            qt = [
                psA_p.tile([128, 512], F32, tag=f"q{q}", name=f"qt{q}")
                for q in range(NQ)
            ]
            for q in range(NQ):
                nc.tensor.matmul(
                    qt[q][:, 0:cw],
                    lq[:, q, :],
                    pr[:, q, c0 : c0 + cw],
                    start=True,
                    stop=True,
                )
            if KSUB == 2:
                continue
            r1x = work_p.tile([128, 512], F32, tag="r1x")
            r2x = work_p.tile([128, 512], F32, tag="r2x")
            r1y = work_p.tile([128, 512], F32, tag="r1y")
            r2y = work_p.tile([128, 512], F32, tag="r2y")
            act.activation(r1x[:, 0:cw], qt[0][:, 0:cw], AF.Relu)
            act.activation(r2x[:, 0:cw], qt[1][:, 0:cw], AF.Relu)
            act.activation(r1y[:, 0:cw], qt[2][:, 0:cw], AF.Relu)
            act.activation(r2y[:, 0:cw], qt[3][:, 0:cw], AF.Relu)
            rs = work_p.tile([128, 512], F32, tag="rs")
            vec.reciprocal_approx_fast(rs[:, 0:cw], qt[4][:, 0:cw])

            if KSUB == 3:
                continue
            mx = work_p.tile([128, 512], F32, tag="mx")
            my = work_p.tile([128, 512], F32, tag="my")
            gps.tensor_tensor(mx[:, 0:cw], r1x[:, 0:cw], r2x[:, 0:cw], OP.min)
            gps.tensor_tensor(my[:, 0:cw], r1y[:, 0:cw], r2y[:, 0:cw], OP.min)
            if KSUB == 4:
                continue
            dxr = work_p.tile([128, 512], F32, tag="dxr")
            dyr = work_p.tile([128, 512], F32, tag="dyr")
            vec.scalar_tensor_tensor(
                dxr[:, 0:cw], mx[:, 0:cw], WT[:, g : g + 1], qt[5][:, 0:cw],
                OP.min, OP.min,
            )
            vec.scalar_tensor_tensor(
                dyr[:, 0:cw], my[:, 0:cw], HT[:, g : g + 1], qt[6][:, 0:cw],
                OP.min, OP.min,
            )
            if KSUB == 5:
                continue
            ip = work_p.tile([128, 512], F32, tag="ip")
            gps.tensor_tensor(ip[:, 0:cw], dxr[:, 0:cw], dyr[:, 0:cw], OP.mult)
            vec.tensor_tensor(S[:, c0 : c0 + cw], ip[:, 0:cw], rs[:, 0:cw], OP.mult)

        vmax = small_p.tile([128, 1], F32, tag="vmax")
        vec.tensor_reduce(vmax[:, :], S[:, :], AX.X, OP.max)

        if STAGE == 2:
            continue

        # argmax (first occurrence of max = jnp.argmax tie-breaking)
        v8 = small_p.tile([128, 8], F32, tag="v8")
        vec.tensor_scalar(v8[:, :], S[:, 0:8], 0.0, vmax[:, :], OP.mult, OP.add)
        idx8 = small_p.tile([128, 8], U32, tag="idx8")
        vec.max_index(idx8[:, :], v8[:, :], S[:, :])
        matchf = small_p.tile([128, 1], F32, tag="matchf")
        vec.tensor_copy(matchf[:, :], idx8[:, 0:1])
        if DEBUG:
            nc.sync.dma_start(dbg_d[:, g : g + 1], matchf[:, :])
            nc.sync.dma_start(dbgs_d[g, :, :], S[:, :])

        if STAGE == 3:
            continue

        # broadcast matched over partitions: transpose -> row -> ones matmul
        mrow_ps = psB_p.tile([1, 128], F32, tag="tp")
        nc.tensor.transpose(mrow_ps[:, :], matchf[:, :], ident[:, :])
        mrow = small_p.tile([1, 128], F32, tag="mrow")
        act.activation(mrow[:, :], mrow_ps[:, :], AF.Copy)
        mb_ps = psB_p.tile([128, 128], F32, tag="tp")
        nc.tensor.matmul(mb_ps[:, :], ones1[:, :], mrow[:, :], start=True, stop=True)
        MB = small_p.tile([128, 128], F32, tag="MB")
        vec.tensor_copy(MB[:, :], mb_ps[:, :])

        # gather rhs: GRB[p, j, col]; cols 0..7 = (nbx1,nby1,bx2,by2) x s,
        # cols 8..15 = logits c x s
        GRB = small_p.tile([128, NJ, 16], F32, tag="GRB")
        for qi, q in enumerate((1, 3, 0, 2)):
            vec.tensor_copy(
                GRB[:, :, 2 * qi : 2 * qi + 2],
                DQ[:, 2 * g : 2 * g + 2, q, :].rearrange("p s j -> p j s"),
            )
        vec.tensor_copy(
            GRB[:, :, 8:16].rearrange("p j (c s) -> p j c s", s=2),
            X[:, 2 * g : 2 * g + 2, :, 5:9].rearrange("p s j c -> p j c s"),
        )

        # one-hot M per j tile (+ per-sample match counts), gather matmuls
        gat = psB_p.tile([128, 16], F32, tag="tp")
        for j in range(NJ):
            M = small_p.tile([128, 128], F32, tag="M")
            for s in range(2):
                vec.tensor_scalar(
                    M[:, 64 * s : 64 * s + 64],
                    MB[:, 64 * s : 64 * s + 64],
                    njcol[:, j : j + 1],
                    None,
                    OP.is_equal,
                    OP.add,
                    accum_out=CNT[:, j, 2 * g + s : 2 * g + s + 1],
                )
            nc.tensor.matmul(
                gat[:, :], M[:, :], GRB[:, j, :], start=(j == 0), stop=(j == NJ - 1)
            )
        vec.tensor_copy(GALL[:, g, :], gat[:, :])

    if STAGE in (2, 3):
        o2 = small_p.tile([3, 1], F32, tag="o2")
        vec.tensor_copy(o2[:, :], TGTC[0:3, 0, 0:1])
        nc.sync.dma_start(out_d[:].rearrange("(x o) -> x o", o=1), o2[:, :])
        return

    # ---------------- losses ----------------
    # box: smooth-l1 on |g - t|; x1/y1 slots hold -x1 so use add there
    junk = big_p.tile([128, BC * NJ], F32, tag="junk")
    D = big_p.tile([128, NG], F32, tag="D")
    DM = big_p.tile([128, NG], F32, tag="DM")
    Q1 = big_p.tile([128, NG], F32, tag="Q1")
    Q2 = big_p.tile([128, NG], F32, tag="Q2")
    col = 0
    for s in range(2):
        P = slice(64 * s, 64 * s + 64)
        for cc, (q2, op_) in enumerate(
            [(0, OP.add), (1, OP.add), (2, OP.subtract), (3, OP.subtract)]
        ):
            vec.tensor_tensor(D[P, :], GALL[P, :, 2 * q2 + s], TGTC[P, :, cc], op_)
            act.activation(D[P, :], D[P, :], AF.Abs)
            vec.tensor_scalar(DM[P, :], D[P, :], 1.0, None, OP.min)
            act.activation(Q1[P, :], DM[P, :], AF.Square, scale=SQRT_HALF)
            act.activation(Q2[P, :], D[P, :], AF.Relu, bias=b_neg1[P, :])
            vec.scalar_tensor_tensor(
                junk[P, 0:NG], Q1[P, :], 0.0, Q2[P, :], OP.add, OP.add,
                accum_out=ACCB[P, col : col + 1],
            )
            col += 1

    # cls: logsumexp(L) - L[y]  (logits ~ N(0,1): no max-subtraction needed)
    Y = big_p.tile([128, NG, C], F32, tag="Y")
    for cc in range(C):
        vec.tensor_scalar(Y[:, :, cc], TCF[:, :], float(cc), None, OP.is_equal)
    E = big_p.tile([128, NG, C], F32, tag="E")
    SE = big_p.tile([128, NG], F32, tag="SE")
    LSE = big_p.tile([128, NG], F32, tag="LSE")
    ZY = big_p.tile([128, NG, C], F32, tag="ZY")
    SZY = big_p.tile([128, NG], F32, tag="SZY")
    for s in range(2):
        P = slice(64 * s, 64 * s + 64)
        L = GALL[P, :, :].rearrange("p g (q two) -> p g q two", two=2)[:, :, 4:8, s]
        act.activation(E[P, :, :], L, AF.Exp)
        vec.tensor_reduce(SE[P, :], E[P, :, :], AX.X, OP.add)
        act.activation(LSE[P, :], SE[P, :], AF.Ln)
        vec.tensor_tensor(ZY[P, :, :], L, Y[P, :, :], OP.mult)
        vec.tensor_reduce(SZY[P, :], ZY[P, :, :], AX.X, OP.add)
        vec.scalar_tensor_tensor(
            junk[P, 0:NG], LSE[P, :], 0.0, SZY[P, :], OP.add, OP.subtract,
            accum_out=ACCC[P, s : s + 1],
        )

    # conf: sum softplus(x) - sum x*pos; softplus = relu(x) + ln(1+exp(-|x|))
    SP = big_p.tile([128, 1], F32, tag="SP")
    SA = big_p.tile([128, BC * NJ], F32, tag="SA")
    SR = big_p.tile([128, BC * NJ], F32, tag="SR")
    x4flat = X[:, :, :, 4].rearrange("p s j -> p (s j)")
    act.activation(SA[:, :], x4flat, AF.Abs)
    act.activation(SA[:, :], SA[:, :], AF.Exp, scale=-1.0)
    act.activation(SA[:, :], SA[:, :], AF.Ln, bias=1.0)
    act.activation(SR[:, :], x4flat, AF.Relu)
    vec.scalar_tensor_tensor(
        junk[:, :], SA[:, :], 0.0, SR[:, :], OP.add, OP.add, accum_out=SP[:, :]
    )
    POS = big_p.tile([128, NJ, BC], F32, tag="POS")
    vec.tensor_scalar(POS[:, :, :], CNT[:, :, :], 1.0, None, OP.is_ge)
    XP = big_p.tile([128, 1], F32, tag="XP")
    vec.scalar_tensor_tensor(
        junk[:, :].rearrange("p (j s) -> p j s", j=NJ),
        POS[:, :, :],
        0.0,
        X[:, :, :, 4].rearrange("p s j -> p j s"),
        OP.add,
        OP.mult,
        accum_out=XP[:, :],
    )

    # combine partials -> [box, cls, conf] via PE partition reduction
    OV = big_p.tile([128, 3], F32, tag="OV")
    vec.tensor_reduce(OV[:, 0:1], ACCB[:, :], AX.X, OP.add)
    vec.tensor_reduce(OV[:, 1:2], ACCC[:, :], AX.X, OP.add)
    vec.tensor_tensor(OV[:, 2:3], SP[:, :], XP[:, :], OP.subtract)
    red_ps = psB_p.tile([3, 1], F32, tag="tp")
    nc.tensor.matmul(red_ps[:, :], OV[:, :], onescol[:, :], start=True, stop=True)
    outs = small_p.tile([3, 1], F32, tag="outs")
    vec.tensor_copy(outs[:, :], red_ps[:, :])
    nc.sync.dma_start(out_d[:].rearrange("(x o) -> x o", o=1), outs[:, :])


_NC = None
TRACE = False
LAST_RESULT = None


def _get_nc():
    global _NC
    if _NC is None:
        _NC = build_kernel()
    return _NC


def _initrows():
    ir = np.zeros((3, NQ * NN + NQ * 128), dtype=np.float32)
    pr = ir[:, : NQ * NN].reshape(3, NQ, NN)
    lq = ir[:, NQ * NN :].reshape(3, NQ, 128)
    for q in range(NQ):
        pr[2, q, :] = QCONST[q]
        lq[0, q, 0:64] = 1.0
        lq[1, q, 64:128] = 1.0
    return ir


def kernel(predictions, target_boxes, target_classes):
    nc = _get_nc()
    ir = _initrows()
    in_maps = []
    for c in range(NCORES):
        sl = slice(c * BC, (c + 1) * BC)
        in_maps.append(
            {
                "predictions": np.ascontiguousarray(predictions[sl]),
                "target_boxes": np.ascontiguousarray(target_boxes[sl]),
                "target_classes": np.ascontiguousarray(target_classes[sl]),
                "initrows": ir,
            }
        )
    global LAST_RESULT
    LAST_RESULT = run_bass_kernel_spmd(
        nc, in_maps, list(range(NCORES)), trace=TRACE
    )
    res = LAST_RESULT.results
    box = np.float64(0.0)
    cls_ = np.float64(0.0)
    conf = np.float64(0.0)
    for c in range(NCORES):
        o = np.asarray(res[c]["out"], dtype=np.float64)
        box += o[0]
        cls_ += o[1]
        conf += o[2]
    total = (5.0 * box + 1.0 * cls_ + conf) / B
    return np.float32(total)

